# revision 22
# baseline (speedup 1.0000x reference)
"""Trainium2 Bass kernel for nn_CausalConvolution (dense_cnn).

Reference computation (B=4, S=4096, H=2048, CIN=COUT=4096, K=4, G=8):
    h   = x @ W_in.T + b_in                       # [B,S,CIN]
    y   = silu(causal_grouped_conv1d(h) + conv_b) # [B,S,COUT], groups=8, k=4
    out = y @ W_out.T + b_out                     # [B,S,H]

Sharding: one conv group per NeuronCore (G = 8 = n_cores).
Core g computes channels [g*512, (g+1)*512) of h (column-parallel W_in),
its conv group (512 in / 512 out channels), and a row-parallel partial of
the output projection. Host sums the 8 partials and adds b_out. No
cross-core communication on device.

All matmuls run in bf16 (fp32 PSUM accumulation); everything is kept in
"transposed" [channel, time] layout on-chip so the contraction dim always
sits on SBUF partitions without any on-chip transposes.

Schedule notes: PE is the bottleneck (6144 N=512 matmuls/core ~= 1.31 ms
at the bf16 streaming limit). Three levers get under it:
 - fp8 DoubleRow on a slice of stage 1: the first 3 hk-pairs of the c=0
   group run as float8e4 DoubleRow matmuls (2 contraction chunks per
   512-cycle pass = 2x rate; operands host-quantized). Adds ~1.3e-2 to
   the rel-err budget (gate 2e-2, measured 1.36e-2 total), saves ~21 us.
 - DMA count minimized: every dma_start serializes ~630 ns through the
   single shared HWDGE block, so x rides a tile-major DRAM layout (one
   16 KB-runs descriptor per time tile, prefetched three tiles ahead on
   the Activation queue) and out partials are stored in bf16 as one
   full-H descriptor per 128-row chunk (4 KB rows) on the SP queue.
   Splitting the final stores (the old schedule used 64 descriptors)
   is what used to cost a ~30 us drain tail.
 - Startup DMAs are issued ungated on the SP queue in consumption-
   deadline order: the engines drain near-FIFO, so issue order IS the
   bandwidth phasing; completion-gating added full-transfer bubbles,
   and the first matmul is data-arrival-bound at ~12 us.
Stage 1 runs one time-tile ahead of stages 2/3 to keep the PE stream
dense; a short warmup matmul burst keeps HAM from throttling the clock
before the stream starts (a >3 us PE gap costs ~2x-slow matmuls after).
"""

import numpy as np
import ml_dtypes

# Problem constants (hardcoded per the harness contract).
B, S, H = 4, 4096, 2048
CIN = COUT = 4096
KT = 4          # conv taps
G = 8           # conv groups == number of cores
CG = CIN // G   # 512 channels per group/core
T = B * S       # 16384 flattened time steps
NCORES = 8

HK = H // 128       # 16 contraction chunks for stage 1
CT = CG // 128      # 4 chunks of the per-core channel dim
TTILE = 512         # time-tile (N of every matmul)
NT = T // TTILE     # 32 time tiles
NH = H // TTILE     # 4 output-column chunks of stage 3

_BF16 = ml_dtypes.bfloat16
_F8 = ml_dtypes.float8_e4m3

_CACHE = {}

# test.py introspection: the most recent BassKernelResults from a run.
LAST_RESULTS = None


def _build_nc():
    import concourse.bass as bass
    import concourse.mybir as mybir
    import concourse.tile as tile
    from concourse.tile import add_dep_helper
    from concourse import bacc

    dt = mybir.dt
    AF = mybir.ActivationFunctionType

    nc = bacc.Bacc(
        "TRN2", target_bir_lowering=False, debug=False, num_devices=NCORES
    )

    # x in tile-major layout: [128, tile, hk, t] so one descriptor per time
    # tile moves 16 KB contiguous runs per partition.
    xT = nc.dram_tensor(
        "xT", [128, NT, HK, TTILE], dt.bfloat16, kind="ExternalInput"
    )
    # fp8 copies of x chunks hk=0..5 and the matching c=0 W_in rows: the
    # c=0 group of stage 1 runs its first 3 hk-pairs as fp8 DoubleRow
    # matmuls (2x PE throughput). Error budget: ~1.3e-2 of the 2e-2 gate.
    x8 = nc.dram_tensor("x8", [128, NT, 6, TTILE], dt.float8e4, kind="ExternalInput")
    w_in = nc.dram_tensor("w_in", [128, CT, HK, 128], dt.bfloat16, kind="ExternalInput")
    w_in8 = nc.dram_tensor("w_in8", [128, 6, 128], dt.float8e4, kind="ExternalInput")
    cw = nc.dram_tensor("cw", [128, KT, CT, CG], dt.bfloat16, kind="ExternalInput")
    wo = nc.dram_tensor("wo", [128, CT, H], dt.bfloat16, kind="ExternalInput")
    b_in = nc.dram_tensor("b_in", [128, CT], dt.float32, kind="ExternalInput")
    cb = nc.dram_tensor("cb", [128, CT], dt.float32, kind="ExternalInput")
    out = nc.dram_tensor("out", [T, H], dt.bfloat16, kind="ExternalOutput")

    n_tt = S // TTILE  # time tiles per batch

    with tile.TileContext(nc) as tc:
        # PE warmup: dep-free matmuls on scratch data run while the first
        # weight/x DMAs are in flight, so HAM un-throttles (K=8/8) before
        # the real matmul stream begins.
        with (
            tc.tile_pool(name="warm", bufs=1) as warmpool,
            tc.tile_pool(name="warmps", bufs=1, space="PSUM") as warmpspool,
        ):
            scratch = warmpool.tile([128, 512], dt.bfloat16)
            nc.vector.memset(scratch[:], 0.0)
            wps = warmpspool.tile([128, 384], dt.float32)
            for _ in range(12):
                nc.tensor.matmul(
                    wps[:], scratch[:, 0:128], scratch[:, 128:512],
                    start=True, stop=True,
                )
        with (
            tc.tile_pool(name="weights", bufs=1) as wpool,
            tc.tile_pool(name="xin", bufs=3) as xpool,
            tc.tile_pool(name="x8in", bufs=3) as x8pool,
            tc.tile_pool(name="hbuf", bufs=2) as hpool,
            tc.tile_pool(name="ybuf", bufs=2) as ypool,
            tc.tile_pool(name="obuf", bufs=2) as opool,
            tc.tile_pool(name="ps1", bufs=2, space="PSUM") as ps1pool,
            tc.tile_pool(name="ps2", bufs=2, space="PSUM") as ps2pool,
            tc.tile_pool(name="ps3", bufs=4, space="PSUM") as ps3pool,
        ):
            # Startup DMA scheduling, one descriptor per tensor chunk (the
            # shared HWDGE serializes ~630 ns per dma_start, so fewer +
            # larger wins). Everything is issued ungated on the SP queue
            # in consumption-deadline order: the DMA engines drain the
            # queue near-FIFO, so ordering the issues IS the bandwidth
            # phasing — completion-gating would add full-transfer bubbles.
            # (The Activation queue can't take the critical first loads:
            # the scalar engine is busy with ACT_TABLE_LOAD until ~8.4us.)
            w_in_sb = wpool.tile([128, CT, HK, 128], dt.bfloat16)
            w_in8_sb = wpool.tile([128, 6, 128], dt.float8e4)
            bin_sb = wpool.tile([128, CT], dt.float32)
            cb_sb = wpool.tile([128, CT], dt.float32)
            cw_sb = wpool.tile([128, KT, CT, CG], dt.bfloat16)
            wo_sb = wpool.tile([128, CT, H], dt.bfloat16)
            xts = {}
            x8ts = {}
            for j in range(3):
                xts[j] = xpool.tile(
                    [128, HK, TTILE], dt.bfloat16, tag="xt", name=f"xt{j}"
                )
                x8ts[j] = x8pool.tile(
                    [128, 6, TTILE], dt.float8e4, tag="x8t", name=f"x8t{j}"
                )
            nc.sync.dma_start(w_in8_sb[:], w_in8[:])
            nc.sync.dma_start(x8ts[0][:], x8[:, 0])
            nc.sync.dma_start(w_in_sb[:, 0], w_in[:, 0])
            nc.sync.dma_start(xts[0][:], xT[:, 0])
            nc.sync.dma_start(bin_sb[:], b_in[:])
            for cc in range(1, CT):
                nc.sync.dma_start(w_in_sb[:, cc], w_in[:, cc])
            nc.sync.dma_start(cb_sb[:], cb[:])
            nc.sync.dma_start(x8ts[1][:], x8[:, 1])
            nc.sync.dma_start(xts[1][:], xT[:, 1])
            nc.sync.dma_start(x8ts[2][:], x8[:, 2])
            nc.sync.dma_start(xts[2][:], xT[:, 2])
            for k in range(KT):
                nc.sync.dma_start(cw_sb[:, k], cw[:, k])
            for oo in range(CT):
                nc.sync.dma_start(wo_sb[:, oo], wo[:, oo])

            tiles = [(b, tt) for b in range(B) for tt in range(n_tt)]
            hts = {}   # batch -> hT tile

            def stage1(b, tt):
                t0 = tt * TTILE
                ti = b * n_tt + tt
                if tt == 0:
                    # h^T for this batch: [c, t] with a 3-column zero halo
                    # in front so causal taps at batch start read zeros.
                    hts[b] = hpool.tile(
                        [128, CT, KT - 1 + S], dt.bfloat16, tag="hT", name="hT"
                    )
                    nc.vector.memset(hts[b][:, :, 0 : KT - 1], 0.0)
                hT = hts[b]
                xt = xts.pop(ti)
                x8t = x8ts.pop(ti)
                for c in range(CT):
                    ps = ps1pool.tile([128, TTILE], dt.float32)
                    if c == 0:
                        # first 3 hk-pairs as fp8 DoubleRow (2 chunks of
                        # the contraction per instruction at 2x rate)
                        for j in range(3):
                            nc.tensor.matmul(
                                ps[:],
                                w_in8_sb[:, 2 * j : 2 * j + 2, :],
                                x8t[:, 2 * j : 2 * j + 2, :],
                                start=(j == 0),
                                stop=False,
                                perf_mode=mybir.MatmulPerfMode.DoubleRow,
                            )
                        hk0 = 6
                    else:
                        hk0 = 0
                    for hk in range(hk0, HK):
                        nc.tensor.matmul(
                            ps[:],
                            w_in_sb[:, c, hk, :],
                            xt[:, hk, :],
                            start=(hk == 0),
                            stop=(hk == HK - 1),
                        )
                    nc.scalar.activation(
                        hT[:, c, KT - 1 + t0 : KT - 1 + t0 + TTILE],
                        ps[:],
                        AF.Identity,
                        bias=bin_sb[:, c : c + 1],
                    )
                # keep the x pipeline three tiles ahead (0..2 preloaded);
                # issued after the c-loop so the WAR wait on the recycled
                # slot (this tile's xt, just freed) never blocks the acts.
                if ti + 3 < NT:
                    xts[ti + 3] = xpool.tile(
                        [128, HK, TTILE], dt.bfloat16, tag="xt", name="xt"
                    )
                    nc.scalar.dma_start(xts[ti + 3][:], xT[:, ti + 3])
                    x8ts[ti + 3] = x8pool.tile(
                        [128, 6, TTILE], dt.float8e4, tag="x8t", name="x8t"
                    )
                    nc.scalar.dma_start(x8ts[ti + 3][:], x8[:, ti + 3])

            def stage23(b, tt):
                t0 = tt * TTILE
                tg = b * S + t0
                hT = hts[b]
                # Stage 2: causal grouped conv as 16 accumulated matmuls
                yt = ypool.tile([128, CT, TTILE], dt.bfloat16, tag="yt")
                for o in range(CT):
                    ps = ps2pool.tile([128, TTILE], dt.float32)
                    n_acc = KT * CT
                    acc = 0
                    for ik in range(CT):
                        for k in range(KT):
                            nc.tensor.matmul(
                                ps[:],
                                cw_sb[:, k, ik, o * 128 : (o + 1) * 128],
                                hT[:, ik, t0 + k : t0 + k + TTILE],
                                start=(acc == 0),
                                stop=(acc == n_acc - 1),
                            )
                            acc += 1
                    nc.scalar.activation(
                        yt[:, o, :],
                        ps[:],
                        AF.Silu,
                        bias=cb_sb[:, o : o + 1],
                    )
                # Stage 3: partial out[t, :] = y^T.T @ W_out_g^T; one store
                # per 128-row chunk (full H width -> 4 KB contiguous rows).
                for ss in range(TTILE // 128):
                    ot = opool.tile([128, H], dt.bfloat16, tag="ot")
                    for nh in range(NH):
                        ps = ps3pool.tile([128, TTILE], dt.float32)
                        for oo in range(CT):
                            nc.tensor.matmul(
                                ps[:],
                                yt[:, oo, ss * 128 : (ss + 1) * 128],
                                wo_sb[:, oo, nh * TTILE : (nh + 1) * TTILE],
                                start=(oo == 0),
                                stop=(oo == CT - 1),
                            )
                        nc.vector.tensor_copy(
                            ot[:, nh * TTILE : (nh + 1) * TTILE], ps[:]
                        )
                    row = tg + ss * 128
                    last = b == B - 1 and tt == n_tt - 1 and ss == TTILE // 128 - 1
                    if last:
                        # the very last store is on the kernel-exit critical
                        # path: halve it across both HWDGE queues
                        nc.sync.dma_start(
                            out[row : row + 128, 0 : H // 2], ot[:, 0 : H // 2]
                        )
                        nc.scalar.dma_start(
                            out[row : row + 128, H // 2 : H], ot[:, H // 2 : H]
                        )
                    else:
                        nc.sync.dma_start(out[row : row + 128, :], ot[:])

            # Stage 1 runs one time-tile ahead of stages 2/3: keeps the PE
            # stream dense and moves the cw/wo DMA deadlines out by a tile.
            for i, (b, tt) in enumerate(tiles):
                stage1(b, tt)
                if i > 0:
                    stage23(*tiles[i - 1])
            stage23(*tiles[-1])

    nc.compile()
    return nc


def _prep_inputs(x, W_in, b_in, conv_w, conv_b, W_out):
    """Host-side shard + transpose + bf16 cast. Returns in_maps for 8 cores."""
    x = np.asarray(x, dtype=np.float32)
    # x^T in tile-major [h_inner=128, tile, h_outer, t] layout
    xr32 = x.reshape(NT, TTILE, HK, 128).transpose(3, 0, 2, 1)  # [128,NT,HK,TTILE]
    xr = np.ascontiguousarray(xr32.astype(_BF16))
    x8r = np.ascontiguousarray(xr32[:, :, 0:6, :].astype(_F8))  # [128, NT, 6, TTILE]

    in_maps = []
    for g in range(NCORES):
        c0 = g * CG
        w_in_f32 = (
            np.asarray(W_in[c0 : c0 + CG, :])
            .reshape(CT, 128, HK, 128)
            .transpose(3, 0, 2, 1)
        )  # [128, CT, HK, 128]: (hi, cc, hk, ci) = W_in[c0+cc*128+ci, hk*128+hi]
        w_in_g = np.ascontiguousarray(w_in_f32.astype(_BF16))
        w_in8_g = np.ascontiguousarray(w_in_f32[:, 0, 0:6, :].astype(_F8))
        cw_g = np.ascontiguousarray(
            np.asarray(conv_w[c0 : c0 + CG, :, :])
            .reshape(CG, CT, 128, KT)
            .transpose(2, 3, 1, 0)
            .astype(_BF16)
        )  # [128, KT, CT, CG]: (ii, k, io, o) = conv_w[c0+o, io*128+ii, k]
        wo_g = np.ascontiguousarray(
            np.asarray(W_out[:, c0 : c0 + CG])
            .reshape(H, CT, 128)
            .transpose(2, 1, 0)
            .astype(_BF16)
        )  # [128, CT, H]: (oi, oo, h) = W_out[h, c0+oo*128+oi]
        bin_g = np.ascontiguousarray(
            np.asarray(b_in[c0 : c0 + CG], dtype=np.float32).reshape(CT, 128).T
        )  # [128, CT]
        cb_g = np.ascontiguousarray(
            np.asarray(conv_b[c0 : c0 + CG], dtype=np.float32).reshape(CT, 128).T
        )
        in_maps.append(
            {
                "xT": xr,
                "x8": x8r,
                "w_in": w_in_g,
                "w_in8": w_in8_g,
                "cw": cw_g,
                "wo": wo_g,
                "b_in": bin_g,
                "cb": cb_g,
            }
        )
    return in_maps


def kernel(x, W_in, b_in, conv_w, conv_b, W_out, b_out):
    global LAST_RESULTS
    from concourse import bass_utils

    if "nc" not in _CACHE:
        _CACHE["nc"] = _build_nc()
    nc = _CACHE["nc"]

    in_maps = _prep_inputs(x, W_in, b_in, conv_w, conv_b, W_out)

    res = bass_utils.run_bass_kernel_spmd(
        nc, in_maps, core_ids=list(range(NCORES))
    )
    LAST_RESULTS = res

    acc = np.asarray(res.results[0]["out"]).astype(np.float32)
    for r in res.results[1:]:
        acc += np.asarray(r["out"]).astype(np.float32)
    acc += np.asarray(b_out, dtype=np.float32)[None, :]
    return acc.reshape(B, S, H)
